# revision 15
# baseline (speedup 1.0000x reference)
import os
import sys

for _p in ("/opt/trn_rl_repo", "/root/.axon_site/_ro/trn_rl_repo"):
    if os.path.isdir(_p) and _p not in sys.path:
        sys.path.insert(0, _p)

import numpy as np
import ml_dtypes

import concourse.bass as bass
import concourse.tile as tile
import concourse.mybir as mybir
from concourse import bacc
from concourse.bass import ts
from concourse.bass_utils import run_bass_kernel_spmd

N_CORES = 8
D = 1024
F = 2048
T = 1024  # tokens per core (8192 / 8)

BF16 = mybir.dt.bfloat16
F32 = mybir.dt.float32


def build(nc, T=T, D=D, F=F, use_silu=True, psg_b=3, psu_b=3, psy_b=2,
          w_b=4, hb_extra=6, xf_b=3):
    """Emit the per-core MoE FFN kernel.

    Layout: activations transposed (feature on partitions, tokens on free dim).
    Paths: [shared, expert0, expert1]; expert token masks folded into the
    input (x0 = x*m0, x1 = x - x0) so all three paths sum directly.
    """
    KD = D // 128   # k-tiles over D (gate/up contraction, also out tiles of down)
    MF = F // 128   # m-tiles over F
    MD = D // 128
    KF = F // 128
    TH = T // 512   # 512-token free-dim blocks

    xt32 = nc.dram_tensor("xt32", [D, T], F32, kind="ExternalInput").ap()
    xtb = nc.dram_tensor("xtb", [D, T], BF16, kind="ExternalInput").ap()
    wr = nc.dram_tensor("wr", [128, KD, 2], F32, kind="ExternalInput").ap()
    rb = nc.dram_tensor("rb", [1, 2], F32, kind="ExternalInput").ap()
    wgl = nc.dram_tensor("wgl", [3 * MF, 128, KD, 128], BF16, kind="ExternalInput").ap()
    wul = nc.dram_tensor("wul", [3 * MF, 128, KD, 128], BF16, kind="ExternalInput").ap()
    wdl = nc.dram_tensor("wdl", [3 * MD, 128, KF, 128], BF16, kind="ExternalInput").ap()
    yt = nc.dram_tensor("yt", [D, T], F32, kind="ExternalOutput").ap()

    with tile.TileContext(nc) as tc:
        with (
            tc.tile_pool(name="xres", bufs=1) as xres,
            tc.tile_pool(name="xf", bufs=xf_b) as xf,
            tc.tile_pool(name="small", bufs=1) as small,
            tc.tile_pool(name="wg", bufs=w_b) as wgp,
            tc.tile_pool(name="wu", bufs=w_b) as wup,
            tc.tile_pool(name="wd", bufs=w_b) as wdp,
            tc.tile_pool(name="hb", bufs=KF + hb_extra) as hb,
            tc.tile_pool(name="gs", bufs=3) as gsp,
            tc.tile_pool(name="yac", bufs=1) as yac,
            tc.tile_pool(name="psg", bufs=psg_b, space="PSUM") as psg,
            tc.tile_pool(name="psu", bufs=psu_b, space="PSUM") as psu,
            tc.tile_pool(name="psy", bufs=psy_b, space="PSUM") as psy,
        ):
            # resident transposed input (bf16) + masked variants
            xtb_sb = xres.tile([128, KD, T], BF16, tag="xtb")
            xtb_r = xtb.rearrange("(ko p) t -> p ko t", p=128)
            for ko in range(KD):
                nc.sync.dma_start(xtb_sb[:, ko, :], xtb_r[:, ko, :])
            x0_sb = xres.tile([128, KD, T], BF16, tag="x0")
            x1_sb = xres.tile([128, KD, T], BF16, tag="x1")

            # ---- router (fp32) ----
            wr_sb = small.tile([128, KD, 2], F32, tag="wr")
            nc.sync.dma_start(wr_sb[:], wr)
            wdiff = small.tile([128, KD, 1], F32, tag="wdiff")
            nc.vector.tensor_sub(wdiff[:], wr_sb[:, :, 0:1], wr_sb[:, :, 1:2])
            rb_sb = small.tile([1, 2], F32, tag="rb")
            nc.sync.dma_start(rb_sb[:], rb)
            bdiff = small.tile([1, 1], F32, tag="bdiff")
            nc.vector.tensor_sub(bdiff[:], rb_sb[:, 0:1], rb_sb[:, 1:2])
            ones_sb = small.tile([1, 128], BF16, tag="ones")
            nc.vector.memset(ones_sb[:], 1.0)
            mask_row = small.tile([1, T], BF16, tag="mrow")
            mask_bc = small.tile([128, T], BF16, tag="mbc")

            prs = [
                psg.tile([1, 512], F32, tag="g", name=f"pr{th}") for th in range(TH)
            ]
            for ko in range(KD):
                xf_t = xf.tile([128, T], F32, tag="xf")
                nc.sync.dma_start(xf_t[:], xt32[ko * 128:(ko + 1) * 128, :])
                for th in range(TH):
                    nc.tensor.matmul(
                        prs[th][:], wdiff[:, ko, :], xf_t[:, ts(th, 512)],
                        start=(ko == 0), stop=(ko == KD - 1),
                    )
            # mask0 = ((l0-l1) + (b0-b1)) >= 0, as 1.0/0.0
            for th in range(TH):
                nc.vector.tensor_scalar(
                    mask_row[:, ts(th, 512)], prs[th][:], bdiff[:], 0.0,
                    mybir.AluOpType.add, mybir.AluOpType.is_ge,
                )
            # broadcast mask row across 128 partitions via K=1 matmul with ones
            for th in range(TH):
                pm = psu.tile([128, 512], F32, tag="u")
                nc.tensor.matmul(
                    pm[:], ones_sb[:], mask_row[:, ts(th, 512)], start=True, stop=True
                )
                nc.vector.tensor_copy(mask_bc[:, ts(th, 512)], pm[:])
            for ko in range(KD):
                nc.vector.tensor_mul(x0_sb[:, ko, :], xtb_sb[:, ko, :], mask_bc[:])
                nc.vector.tensor_sub(x1_sb[:, ko, :], xtb_sb[:, ko, :], x0_sb[:, ko, :])

            # ---- 3 SwiGLU paths ----
            yt_r = yt.rearrange("(md p) t -> p md t", p=128)
            yaccs = [
                yac.tile([128, T], F32, tag=f"yacc{md}", name=f"yacc{md}")
                for md in range(MD)
            ]
            xs_by_path = [xtb_sb, x0_sb, x1_sb]
            for p in range(3):
                xp = xs_by_path[p]
                hch = []
                for mf in range(MF):
                    wg_t = wgp.tile([128, KD, 128], BF16, tag="wg")
                    nc.sync.dma_start(wg_t[:], wgl[p * MF + mf])
                    wu_t = wup.tile([128, KD, 128], BF16, tag="wu")
                    nc.sync.dma_start(wu_t[:], wul[p * MF + mf])
                    h_t = hb.tile([128, T], BF16, tag="h")
                    pgs = [
                        psg.tile([128, 512], F32, tag="g", name=f"pg{th}")
                        for th in range(TH)
                    ]
                    pus = [
                        psu.tile([128, 512], F32, tag="u", name=f"pu{th}")
                        for th in range(TH)
                    ]
                    for th in range(TH):
                        for ko in range(KD):
                            nc.tensor.matmul(
                                pgs[th][:], wg_t[:, ko, :], xp[:, ko, ts(th, 512)],
                                start=(ko == 0), stop=(ko == KD - 1),
                            )
                        for ko in range(KD):
                            nc.tensor.matmul(
                                pus[th][:], wu_t[:, ko, :], xp[:, ko, ts(th, 512)],
                                start=(ko == 0), stop=(ko == KD - 1),
                            )
                    for th in range(TH):
                        pg, pu = pgs[th], pus[th]
                        g_s = gsp.tile([128, 512], BF16, tag="gs")
                        if use_silu:
                            nc.scalar.activation(
                                g_s[:], pg[:], mybir.ActivationFunctionType.Silu
                            )
                        else:
                            # CoreSim lacks Silu; g*sigmoid(g) is identical math
                            nc.scalar.activation(
                                g_s[:], pg[:], mybir.ActivationFunctionType.Sigmoid
                            )
                            nc.vector.tensor_mul(g_s[:], g_s[:], pg[:])
                        nc.vector.tensor_mul(h_t[:, ts(th, 512)], g_s[:], pu[:])
                    hch.append(h_t)
                for md in range(MD):
                    wd_t = wdp.tile([128, KF, 128], BF16, tag="wd")
                    nc.sync.dma_start(wd_t[:], wdl[p * MD + md])
                    pys = [
                        psy.tile([128, 512], F32, tag="y", name=f"py{th}")
                        for th in range(TH)
                    ]
                    for kf in range(KF):
                        for th in range(TH):
                            nc.tensor.matmul(
                                pys[th][:], wd_t[:, kf, :], hch[kf][:, ts(th, 512)],
                                start=(kf == 0), stop=(kf == KF - 1),
                            )
                    for th in range(TH):
                        if p == 0:
                            nc.vector.tensor_copy(
                                yaccs[md][:, ts(th, 512)], pys[th][:]
                            )
                        else:
                            nc.vector.tensor_add(
                                yaccs[md][:, ts(th, 512)],
                                yaccs[md][:, ts(th, 512)],
                                pys[th][:],
                            )
                    if p == 2:
                        # final path: this md slice is complete, ship it out
                        nc.sync.dma_start(yt_r[:, md, :], yaccs[md][:])
    return nc


def pack_inputs(x, W_router, router_bias, Wg, Wu, Wd, Sg, Su, Sd, T=T, D=D, F=F):
    """Host-side sharding + layout prep. Returns per-core in_maps."""
    KD, MF, MD, KF = D // 128, F // 128, D // 128, F // 128
    flat = np.asarray(x, np.float32).reshape(-1, D)
    n_tokens = flat.shape[0]
    assert n_tokens == N_CORES * T
    xt = np.ascontiguousarray(flat.T)  # [D, N]
    xtb_full = xt.astype(ml_dtypes.bfloat16)

    G = np.stack([np.asarray(Sg), np.asarray(Wg)[0], np.asarray(Wg)[1]]).astype(np.float32)
    U = np.stack([np.asarray(Su), np.asarray(Wu)[0], np.asarray(Wu)[1]]).astype(np.float32)
    Dn = np.stack([np.asarray(Sd), np.asarray(Wd)[0], np.asarray(Wd)[1]]).astype(np.float32)
    wgl = np.ascontiguousarray(
        G.reshape(3, KD, 128, MF, 128).transpose(0, 3, 2, 1, 4)
    ).reshape(3 * MF, 128, KD, 128).astype(ml_dtypes.bfloat16)
    wul = np.ascontiguousarray(
        U.reshape(3, KD, 128, MF, 128).transpose(0, 3, 2, 1, 4)
    ).reshape(3 * MF, 128, KD, 128).astype(ml_dtypes.bfloat16)
    wdl = np.ascontiguousarray(
        Dn.reshape(3, KF, 128, MD, 128).transpose(0, 3, 2, 1, 4)
    ).reshape(3 * MD, 128, KF, 128).astype(ml_dtypes.bfloat16)
    wr_h = np.ascontiguousarray(
        np.asarray(W_router, np.float32).reshape(KD, 128, 2).transpose(1, 0, 2)
    )
    rb_h = np.asarray(router_bias, np.float32).reshape(1, 2)

    in_maps = []
    for c in range(N_CORES):
        sl = slice(c * T, (c + 1) * T)
        in_maps.append({
            "xt32": np.ascontiguousarray(xt[:, sl]),
            "xtb": np.ascontiguousarray(xtb_full[:, sl]),
            "wr": wr_h,
            "rb": rb_h,
            "wgl": wgl,
            "wul": wul,
            "wdl": wdl,
        })
    return in_maps


_CACHE = {}


def _get_compiled():
    if "nc" not in _CACHE:
        nc = bacc.Bacc(
            "TRN2", target_bir_lowering=False, debug=False, num_devices=N_CORES
        )
        build(nc)
        nc.compile()
        _CACHE["nc"] = nc
    return _CACHE["nc"]


def kernel(x, W_router, router_bias, Wg, Wu, Wd, Sg, Su, Sd, _trace=False, **_kw):
    nc = _get_compiled()
    in_maps = pack_inputs(x, W_router, router_bias, Wg, Wu, Wd, Sg, Su, Sd)
    res = run_bass_kernel_spmd(
        nc, in_maps, core_ids=list(range(N_CORES)), trace=_trace
    )
    out_t = np.concatenate(
        [res.results[c]["yt"] for c in range(N_CORES)], axis=1
    )  # [D, N]
    out = np.ascontiguousarray(out_t.T).reshape(np.asarray(x).shape)
    if _trace:
        _CACHE["last_result"] = res
    return out.astype(np.float32)


# revision 25
# speedup vs baseline: 1.2052x; 1.2052x over previous
import os
import sys

for _p in ("/opt/trn_rl_repo", "/root/.axon_site/_ro/trn_rl_repo"):
    if os.path.isdir(_p) and _p not in sys.path:
        sys.path.insert(0, _p)

import numpy as np
import ml_dtypes

import concourse.bass as bass
import concourse.tile as tile
import concourse.mybir as mybir
from concourse import bacc
from concourse._compat import axon_active
from concourse.bass import ts
from concourse.bass_utils import run_bass_kernel_spmd
from concourse.masks import make_identity

N_CORES = 8
D = 1024
F = 2048
T = 1024  # tokens per core (8192 / 8)

BF16 = mybir.dt.bfloat16
F32 = mybir.dt.float32


def build(nc, T=T, D=D, F=F, use_silu=True, psg_b=3, psu_b=3, psy_b=2,
          w_b=4, hb_extra=6, xf_b=3):
    """Emit the per-core MoE FFN kernel.

    Layout: activations transposed (feature on partitions, tokens on free dim).
    Paths: [shared, expert0, expert1]; expert token masks folded into the
    input (x0 = x*m0, x1 = x - x0) so all three paths sum directly.
    """
    KD = D // 128   # k-tiles over D (gate/up contraction, also out tiles of down)
    MF = F // 128   # m-tiles over F
    MD = D // 128
    KF = F // 128
    TH = T // 512   # 512-token free-dim blocks

    xt32 = nc.dram_tensor("xt32", [D, T], F32, kind="ExternalInput").ap()
    xtb = nc.dram_tensor("xtb", [D, T], BF16, kind="ExternalInput").ap()
    wr = nc.dram_tensor("wr", [128, KD, 2], F32, kind="ExternalInput").ap()
    rb = nc.dram_tensor("rb", [1, 2], F32, kind="ExternalInput").ap()
    wgl = nc.dram_tensor("wgl", [3 * MF, 128, KD, 128], BF16, kind="ExternalInput").ap()
    wul = nc.dram_tensor("wul", [3 * MF, 128, KD, 128], BF16, kind="ExternalInput").ap()
    wdl = nc.dram_tensor("wdl", [3 * MD, 128, KF, 128], BF16, kind="ExternalInput").ap()
    yt = nc.dram_tensor("yt", [D, T], F32, kind="ExternalOutput").ap()

    with tile.TileContext(nc) as tc:
        with (
            tc.tile_pool(name="xres", bufs=1) as xres,
            tc.tile_pool(name="xf", bufs=xf_b) as xf,
            tc.tile_pool(name="small", bufs=1) as small,
            tc.tile_pool(name="wg", bufs=w_b) as wgp,
            tc.tile_pool(name="wu", bufs=w_b) as wup,
            tc.tile_pool(name="wd", bufs=w_b) as wdp,
            tc.tile_pool(name="hb", bufs=KF + hb_extra) as hb,
            tc.tile_pool(name="gs", bufs=3) as gsp,
            tc.tile_pool(name="yac", bufs=1) as yac,
            tc.tile_pool(name="psg", bufs=psg_b, space="PSUM") as psg,
            tc.tile_pool(name="psu", bufs=psu_b, space="PSUM") as psu,
            tc.tile_pool(name="psy", bufs=psy_b, space="PSUM") as psy,
        ):
            # resident transposed input (bf16) + masked variants
            xtb_sb = xres.tile([128, KD, T], BF16, tag="xtb")
            xtb_r = xtb.rearrange("(ko p) t -> p ko t", p=128)
            for ko in range(KD):
                nc.sync.dma_start(xtb_sb[:, ko, :], xtb_r[:, ko, :])
            x0_sb = xres.tile([128, KD, T], BF16, tag="x0")
            x1_sb = xres.tile([128, KD, T], BF16, tag="x1")

            # ---- router (fp32) ----
            wr_sb = small.tile([128, KD, 2], F32, tag="wr")
            nc.sync.dma_start(wr_sb[:], wr)
            wdiff = small.tile([128, KD, 1], F32, tag="wdiff")
            nc.vector.tensor_sub(wdiff[:], wr_sb[:, :, 0:1], wr_sb[:, :, 1:2])
            rb_sb = small.tile([1, 2], F32, tag="rb")
            nc.sync.dma_start(rb_sb[:], rb)
            bdiff = small.tile([1, 1], F32, tag="bdiff")
            nc.vector.tensor_sub(bdiff[:], rb_sb[:, 0:1], rb_sb[:, 1:2])
            ones_sb = small.tile([1, 128], BF16, tag="ones")
            nc.vector.memset(ones_sb[:], 1.0)
            mask_row = small.tile([1, T], BF16, tag="mrow")
            mask_bc = small.tile([128, T], BF16, tag="mbc")

            prs = [
                psg.tile([1, 512], F32, tag="g", name=f"pr{th}") for th in range(TH)
            ]
            for ko in range(KD):
                xf_t = xf.tile([128, T], F32, tag="xf")
                nc.sync.dma_start(xf_t[:], xt32[ko * 128:(ko + 1) * 128, :])
                for th in range(TH):
                    nc.tensor.matmul(
                        prs[th][:], wdiff[:, ko, :], xf_t[:, ts(th, 512)],
                        start=(ko == 0), stop=(ko == KD - 1),
                    )
            # mask0 = ((l0-l1) + (b0-b1)) >= 0, as 1.0/0.0
            for th in range(TH):
                nc.vector.tensor_scalar(
                    mask_row[:, ts(th, 512)], prs[th][:], bdiff[:], 0.0,
                    mybir.AluOpType.add, mybir.AluOpType.is_ge,
                )
            # broadcast mask row across 128 partitions via K=1 matmul with ones
            for th in range(TH):
                pm = psu.tile([128, 512], F32, tag="u")
                nc.tensor.matmul(
                    pm[:], ones_sb[:], mask_row[:, ts(th, 512)], start=True, stop=True
                )
                nc.vector.tensor_copy(mask_bc[:, ts(th, 512)], pm[:])
            for ko in range(KD):
                nc.vector.tensor_mul(x0_sb[:, ko, :], xtb_sb[:, ko, :], mask_bc[:])
                nc.vector.tensor_sub(x1_sb[:, ko, :], xtb_sb[:, ko, :], x0_sb[:, ko, :])

            # ---- 3 SwiGLU paths ----
            yt_r = yt.rearrange("(md p) t -> p md t", p=128)
            yaccs = [
                yac.tile([128, T], F32, tag=f"yacc{md}", name=f"yacc{md}")
                for md in range(MD)
            ]
            xs_by_path = [xtb_sb, x0_sb, x1_sb]
            for p in range(3):
                xp = xs_by_path[p]
                hch = []
                for mf in range(MF):
                    wg_t = wgp.tile([128, KD, 128], BF16, tag="wg")
                    nc.sync.dma_start(wg_t[:], wgl[p * MF + mf])
                    wu_t = wup.tile([128, KD, 128], BF16, tag="wu")
                    nc.sync.dma_start(wu_t[:], wul[p * MF + mf])
                    h_t = hb.tile([128, T], BF16, tag="h")
                    pgs = [
                        psg.tile([128, 512], F32, tag="g", name=f"pg{th}")
                        for th in range(TH)
                    ]
                    pus = [
                        psu.tile([128, 512], F32, tag="u", name=f"pu{th}")
                        for th in range(TH)
                    ]
                    for th in range(TH):
                        for ko in range(KD):
                            nc.tensor.matmul(
                                pgs[th][:], wg_t[:, ko, :], xp[:, ko, ts(th, 512)],
                                start=(ko == 0), stop=(ko == KD - 1),
                            )
                        for ko in range(KD):
                            nc.tensor.matmul(
                                pus[th][:], wu_t[:, ko, :], xp[:, ko, ts(th, 512)],
                                start=(ko == 0), stop=(ko == KD - 1),
                            )
                    for th in range(TH):
                        pg, pu = pgs[th], pus[th]
                        g_s = gsp.tile([128, 512], BF16, tag="gs")
                        if use_silu:
                            nc.scalar.activation(
                                g_s[:], pg[:], mybir.ActivationFunctionType.Silu
                            )
                        else:
                            # CoreSim lacks Silu; g*sigmoid(g) is identical math
                            nc.scalar.activation(
                                g_s[:], pg[:], mybir.ActivationFunctionType.Sigmoid
                            )
                            nc.vector.tensor_mul(g_s[:], g_s[:], pg[:])
                        nc.vector.tensor_mul(h_t[:, ts(th, 512)], g_s[:], pu[:])
                    hch.append(h_t)
                for md in range(MD):
                    wd_t = wdp.tile([128, KF, 128], BF16, tag="wd")
                    nc.sync.dma_start(wd_t[:], wdl[p * MD + md])
                    pys = [
                        psy.tile([128, 512], F32, tag="y", name=f"py{th}")
                        for th in range(TH)
                    ]
                    for kf in range(KF):
                        for th in range(TH):
                            nc.tensor.matmul(
                                pys[th][:], wd_t[:, kf, :], hch[kf][:, ts(th, 512)],
                                start=(kf == 0), stop=(kf == KF - 1),
                            )
                    for th in range(TH):
                        if p == 0:
                            nc.vector.tensor_copy(
                                yaccs[md][:, ts(th, 512)], pys[th][:]
                            )
                        else:
                            nc.vector.tensor_add(
                                yaccs[md][:, ts(th, 512)],
                                yaccs[md][:, ts(th, 512)],
                                pys[th][:],
                            )
                    if p == 2:
                        # final path: this md slice is complete, ship it out
                        nc.sync.dma_start(yt_r[:, md, :], yaccs[md][:])
    return nc


def build_v2(nc, T=T, D=D, F=F, W=128, use_silu=True,
             psg_b=3, psu_b=2, psy_b=3, w_b=4, hb_extra=2, xf_b=3):
    """Token-sorted variant: sort tokens by routed expert (permutation-matrix
    matmul), run expert0 on sorted block [0, T/2) and expert1 on [T/2, T)
    unmasked, and fix the misassigned span around T/2 with a signed-mask
    correction window of W tokens. Exports dst (sort positions) and c0
    (expert-0 count) so the host can unpermute / verify window coverage.
    """
    KD = D // 128   # k-tiles over D
    MF = F // 128
    MD = D // 128
    KF = F // 128
    TH = T // 512   # 512-token blocks over all tokens (shared path)
    TT = T // 128   # 128-token tiles (for P build / gather)
    half = T // 2
    HB = max(1, half // 512)
    HF = half // HB          # free dim of expert block matmuls (<=512)
    w0 = half - W // 2

    xt32 = nc.dram_tensor("xt32", [D, T], F32, kind="ExternalInput").ap()
    xtok = nc.dram_tensor("xtok", [T, D], BF16, kind="ExternalInput").ap()
    iota = nc.dram_tensor("iota", [1, T], F32, kind="ExternalInput").ap()
    wr = nc.dram_tensor("wr", [128, KD, 2], F32, kind="ExternalInput").ap()
    rb = nc.dram_tensor("rb", [1, 2], F32, kind="ExternalInput").ap()
    wgl = nc.dram_tensor("wgl", [3 * MF, 128, KD, 128], BF16, kind="ExternalInput").ap()
    wul = nc.dram_tensor("wul", [3 * MF, 128, KD, 128], BF16, kind="ExternalInput").ap()
    wdl = nc.dram_tensor("wdl", [3 * MD, 128, KF, 128], BF16, kind="ExternalInput").ap()
    yt = nc.dram_tensor("yt", [D, T], F32, kind="ExternalOutput").ap()
    dst = nc.dram_tensor("dst", [1, T], F32, kind="ExternalOutput").ap()
    c0o = nc.dram_tensor("c0o", [1, 1], F32, kind="ExternalOutput").ap()

    AF = mybir.ActivationFunctionType

    with tile.TileContext(nc) as tc:
        with (
            tc.tile_pool(name="xres", bufs=1) as xres,
            tc.tile_pool(name="sigp", bufs=1) as sigp,
            tc.tile_pool(name="psg", bufs=psg_b, space="PSUM") as psg,
            tc.tile_pool(name="psu", bufs=psu_b, space="PSUM") as psu,
            tc.tile_pool(name="psy", bufs=psy_b, space="PSUM") as psy,
        ):
          with (
            tc.tile_pool(name="xtokp", bufs=1) as xtokp,
            tc.tile_pool(name="xf", bufs=xf_b) as xf,
            tc.tile_pool(name="small", bufs=1) as small,
            tc.tile_pool(name="scr", bufs=3) as scr,
            tc.tile_pool(name="pp", bufs=TT) as pp,
          ):
            # token-major x strips (gather lhsT)
            xtok_sb = xtokp.tile([128, TT, D], BF16, tag="xtok")
            for tt in range(TT):
                nc.sync.dma_start(
                    xtok_sb[:, tt, :], xtok[tt * 128:(tt + 1) * 128, :]
                )
            xs_sb = xres.tile([128, KD, T], BF16, tag="xs")  # sorted x^T

            # ---- router (fp32), identical to v1 ----
            wr_sb = small.tile([128, KD, 2], F32, tag="wr")
            nc.sync.dma_start(wr_sb[:], wr)
            wdiff = small.tile([128, KD, 1], F32, tag="wdiff")
            nc.vector.tensor_sub(wdiff[:], wr_sb[:, :, 0:1], wr_sb[:, :, 1:2])
            rb_sb = small.tile([1, 2], F32, tag="rb")
            nc.sync.dma_start(rb_sb[:], rb)
            bdiff = small.tile([1, 1], F32, tag="bdiff")
            nc.vector.tensor_sub(bdiff[:], rb_sb[:, 0:1], rb_sb[:, 1:2])
            mask_row = small.tile([1, T], BF16, tag="mrow")

            prs = [
                psg.tile([1, 512], F32, tag="g", name=f"pr{th}") for th in range(TH)
            ]
            for ko in range(KD):
                xf_t = xf.tile([128, T], F32, tag="xf")
                nc.sync.dma_start(xf_t[:], xt32[ko * 128:(ko + 1) * 128, :])
                for th in range(TH):
                    nc.tensor.matmul(
                        prs[th][:], wdiff[:, ko, :], xf_t[:, ts(th, 512)],
                        start=(ko == 0), stop=(ko == KD - 1),
                    )
            for th in range(TH):
                nc.vector.tensor_scalar(
                    mask_row[:, ts(th, 512)], prs[th][:], bdiff[:], 0.0,
                    mybir.AluOpType.add, mybir.AluOpType.is_ge,
                )

            # ---- sort metadata: dest position per token ----
            iota_sb = small.tile([1, T], F32, tag="iota")
            nc.sync.dma_start(iota_sb[:], iota)
            m32 = small.tile([1, T], F32, tag="m32")
            nc.vector.tensor_copy(m32[:], mask_row[:])
            c0t = small.tile([1, 1], F32, tag="c0t")
            nc.vector.tensor_reduce(
                c0t[:], m32[:], mybir.AxisListType.X, mybir.AluOpType.add
            )
            nc.sync.dma_start(c0o, c0t[:])
            zrow = scr.tile([1, T], F32, tag="sc", name="zrow")
            nc.vector.memset(zrow[:], 0.0)
            srow = small.tile([1, T], F32, tag="srow")
            nc.vector.tensor_tensor_scan(
                srow[:], m32[:], zrow[:], 0.0,
                mybir.AluOpType.add, mybir.AluOpType.add,
            )
            # dest = m*(s-1) + (1-m)*(c0 + t - s) = B + m*(A - B)
            t1 = scr.tile([1, T], F32, tag="sc", name="t1")
            nc.vector.tensor_sub(t1[:], iota_sb[:], srow[:])
            nc.vector.tensor_scalar_add(t1[:], t1[:], c0t[:])        # B
            t2 = scr.tile([1, T], F32, tag="sc", name="t2")
            nc.vector.tensor_scalar_sub(t2[:], srow[:], 1.0)         # A
            nc.vector.tensor_sub(t2[:], t2[:], t1[:])                # A-B
            nc.vector.tensor_mul(t2[:], t2[:], m32[:])               # m*(A-B)
            dtile = small.tile([128, T], F32, tag="dtile")
            nc.vector.memset(dtile[:], 0.0)
            nc.vector.tensor_add(dtile[0:1, :], t1[:], t2[:])        # dest row
            nc.sync.dma_start(dst, dtile[0:1, :])

            # ---- dest row -> per-partition columns (PE transpose) ----
            ident = small.tile([128, 128], F32, tag="ident")
            make_identity(nc, ident[:])
            dcol = small.tile([128, TT], F32, tag="dcol")
            for tt in range(TT):
                ptp = psg.tile([128, 128], F32, tag="g", name=f"ptp{tt}")
                nc.tensor.transpose(ptp[:], dtile[:, ts(tt, 128)], ident[:])
                nc.vector.tensor_copy(dcol[:, tt:tt + 1], ptp[:, 0:1])

            # ---- iota broadcast across partitions ----
            ones_f = small.tile([1, 128], F32, tag="onesf")
            nc.vector.memset(ones_f[:], 1.0)
            iota128 = small.tile([128, T], F32, tag="iota128")
            for th in range(TH):
                pm = psu.tile([128, 512], F32, tag="u", name=f"pio{th}")
                nc.tensor.matmul(
                    pm[:], ones_f[:], iota_sb[:, ts(th, 512)], start=True, stop=True
                )
                nc.vector.tensor_copy(iota128[:, ts(th, 512)], pm[:])

            # ---- permutation tiles + gather matmuls: xs = x_tok^T @ P ----
            ptiles = []
            for tt in range(TT):
                p_t = pp.tile([128, T], BF16, tag="p", name=f"P{tt}")
                nc.vector.tensor_scalar(
                    p_t[:], iota128[:], dcol[:, tt:tt + 1], None,
                    mybir.AluOpType.is_equal,
                )
                ptiles.append(p_t)
            for dt in range(KD):
                for th in range(TH):
                    px = psg.tile([128, 512], F32, tag="g", name=f"px{dt}_{th}")
                    for tt in range(TT):
                        nc.tensor.matmul(
                            px[:], xtok_sb[:, tt, ts(dt, 128)],
                            ptiles[tt][:, ts(th, 512)],
                            start=(tt == 0), stop=(tt == TT - 1),
                        )
                    nc.vector.tensor_copy(xs_sb[:, dt, ts(th, 512)], px[:])

            # ---- correction-window signed masks ----
            siga = scr.tile([1, W], F32, tag="sw", name="siga")
            nc.vector.tensor_scalar(
                siga[:], iota_sb[:, w0:w0 + W], c0t[:], None, mybir.AluOpType.is_ge
            )
            sigb = scr.tile([1, W], F32, tag="sw", name="sigb")
            nc.vector.tensor_scalar(
                sigb[:], iota_sb[:, w0:w0 + W], float(half), None,
                mybir.AluOpType.is_ge,
            )
            sigr = scr.tile([1, W], F32, tag="sw", name="sigr")
            nc.vector.tensor_sub(sigr[:], sigb[:], siga[:])   # +/-1/0 for E0 part
            sig_bc = sigp.tile([128, W], BF16, tag="sigbc")
            sgn_bc = sigp.tile([128, W], BF16, tag="sgnbc")
            psig = psu.tile([128, W], F32, tag="u", name="psig")
            nc.tensor.matmul(psig[:], ones_f[:], sigr[:], start=True, stop=True)
            nc.vector.tensor_copy(sig_bc[:], psig[:])
            nc.vector.tensor_scalar_mul(sgn_bc[:], sig_bc[:], -1.0)

          # ---- paths (sort-phase pools closed; open main-phase pools) ----
          with (
            tc.tile_pool(name="wg", bufs=w_b) as wgp,
            tc.tile_pool(name="wu", bufs=w_b) as wup,
            tc.tile_pool(name="wd", bufs=w_b) as wdp,
            tc.tile_pool(name="hb", bufs=KF + hb_extra) as hb,
            tc.tile_pool(name="hh", bufs=KF + 1) as hhp,
            tc.tile_pool(name="hw", bufs=KF + 1) as hwp,
            tc.tile_pool(name="gs", bufs=3) as gsp,
            tc.tile_pool(name="yac", bufs=1) as yac,
          ):
            yt_r = yt.rearrange("(md p) t -> p md t", p=128)
            yaccs = [
                yac.tile([128, T], F32, tag=f"yacc{md}", name=f"yacc{md}")
                for md in range(MD)
            ]

            def silu_into(dstp, psrc, wdt):
                g_s = gsp.tile([128, wdt], BF16, tag="gs", name="g_s")
                if use_silu:
                    nc.scalar.activation(g_s[:], psrc[:], AF.Silu)
                else:
                    nc.scalar.activation(g_s[:], psrc[:], AF.Sigmoid)
                    nc.vector.tensor_mul(g_s[:], g_s[:], psrc[:])
                return g_s

            # shared path over all (sorted) tokens
            hch = []
            for mf in range(MF):
                wg_t = wgp.tile([128, KD, 128], BF16, tag="wg")
                nc.sync.dma_start(wg_t[:], wgl[mf])
                wu_t = wup.tile([128, KD, 128], BF16, tag="wu")
                nc.sync.dma_start(wu_t[:], wul[mf])
                h_t = hb.tile([128, T], BF16, tag="h")
                for th in range(TH):
                    pg = psg.tile([128, 512], F32, tag="g")
                    pu = psu.tile([128, 512], F32, tag="u")
                    for ko in range(KD):
                        nc.tensor.matmul(
                            pg[:], wg_t[:, ko, :], xs_sb[:, ko, ts(th, 512)],
                            start=(ko == 0), stop=(ko == KD - 1),
                        )
                    for ko in range(KD):
                        nc.tensor.matmul(
                            pu[:], wu_t[:, ko, :], xs_sb[:, ko, ts(th, 512)],
                            start=(ko == 0), stop=(ko == KD - 1),
                        )
                    g_s = silu_into(h_t, pg, 512)
                    nc.vector.tensor_mul(h_t[:, ts(th, 512)], g_s[:], pu[:])
                hch.append(h_t)
            for md in range(MD):
                wd_t = wdp.tile([128, KF, 128], BF16, tag="wd")
                nc.sync.dma_start(wd_t[:], wdl[md])
                for th in range(TH):
                    py = psy.tile([128, 512], F32, tag="y")
                    for kf in range(KF):
                        nc.tensor.matmul(
                            py[:], wd_t[:, kf, :], hch[kf][:, ts(th, 512)],
                            start=(kf == 0), stop=(kf == KF - 1),
                        )
                    nc.vector.tensor_copy(yaccs[md][:, ts(th, 512)], py[:])

            # expert blocks + correction window
            for e in (1, 2):
                off = 0 if e == 1 else half
                wmask = sig_bc if e == 1 else sgn_bc
                hA = []
                hW = []
                for mf in range(MF):
                    wg_t = wgp.tile([128, KD, 128], BF16, tag="wg")
                    nc.sync.dma_start(wg_t[:], wgl[e * MF + mf])
                    wu_t = wup.tile([128, KD, 128], BF16, tag="wu")
                    nc.sync.dma_start(wu_t[:], wul[e * MF + mf])
                    hA_t = hhp.tile([128, half], BF16, tag="hh")
                    for hbk in range(HB):
                        o2 = off + hbk * HF
                        pg = psg.tile([128, HF], F32, tag="g")
                        pu = psu.tile([128, HF], F32, tag="u")
                        for ko in range(KD):
                            nc.tensor.matmul(
                                pg[:], wg_t[:, ko, :], xs_sb[:, ko, o2:o2 + HF],
                                start=(ko == 0), stop=(ko == KD - 1),
                            )
                        for ko in range(KD):
                            nc.tensor.matmul(
                                pu[:], wu_t[:, ko, :], xs_sb[:, ko, o2:o2 + HF],
                                start=(ko == 0), stop=(ko == KD - 1),
                            )
                        g_s = silu_into(hA_t, pg, HF)
                        nc.vector.tensor_mul(
                            hA_t[:, hbk * HF:(hbk + 1) * HF], g_s[:], pu[:]
                        )
                    # correction window with this expert's weights
                    hW_t = hwp.tile([128, W], BF16, tag="hw")
                    pgw = psg.tile([128, W], F32, tag="g", name="pgw")
                    puw = psu.tile([128, W], F32, tag="u", name="puw")
                    for ko in range(KD):
                        nc.tensor.matmul(
                            pgw[:], wg_t[:, ko, :], xs_sb[:, ko, w0:w0 + W],
                            start=(ko == 0), stop=(ko == KD - 1),
                        )
                    for ko in range(KD):
                        nc.tensor.matmul(
                            puw[:], wu_t[:, ko, :], xs_sb[:, ko, w0:w0 + W],
                            start=(ko == 0), stop=(ko == KD - 1),
                        )
                    g_s = silu_into(hW_t, pgw, W)
                    nc.vector.tensor_mul(hW_t[:], g_s[:], puw[:])
                    nc.vector.tensor_mul(hW_t[:], hW_t[:], wmask[:])
                    hA.append(hA_t)
                    hW.append(hW_t)
                for md in range(MD):
                    wd_t = wdp.tile([128, KF, 128], BF16, tag="wd")
                    nc.sync.dma_start(wd_t[:], wdl[e * MD + md])
                    for hbk in range(HB):
                        o2 = off + hbk * HF
                        py = psy.tile([128, HF], F32, tag="y")
                        for kf in range(KF):
                            nc.tensor.matmul(
                                py[:], wd_t[:, kf, :],
                                hA[kf][:, hbk * HF:(hbk + 1) * HF],
                                start=(kf == 0), stop=(kf == KF - 1),
                            )
                        nc.vector.tensor_add(
                            yaccs[md][:, o2:o2 + HF],
                            yaccs[md][:, o2:o2 + HF], py[:],
                        )
                    pyw = psy.tile([128, W], F32, tag="y", name="pyw")
                    for kf in range(KF):
                        nc.tensor.matmul(
                            pyw[:], wd_t[:, kf, :], hW[kf][:],
                            start=(kf == 0), stop=(kf == KF - 1),
                        )
                    nc.vector.tensor_add(
                        yaccs[md][:, w0:w0 + W],
                        yaccs[md][:, w0:w0 + W], pyw[:],
                    )
                    if e == 2:
                        nc.sync.dma_start(yt_r[:, md, :], yaccs[md][:])
    return nc


def pack_inputs(x, W_router, router_bias, Wg, Wu, Wd, Sg, Su, Sd, T=T, D=D, F=F):
    """Host-side sharding + layout prep. Returns per-core in_maps."""
    KD, MF, MD, KF = D // 128, F // 128, D // 128, F // 128
    flat = np.asarray(x, np.float32).reshape(-1, D)
    n_tokens = flat.shape[0]
    assert n_tokens == N_CORES * T
    xt = np.ascontiguousarray(flat.T)  # [D, N]
    xtb_full = xt.astype(ml_dtypes.bfloat16)

    G = np.stack([np.asarray(Sg), np.asarray(Wg)[0], np.asarray(Wg)[1]]).astype(np.float32)
    U = np.stack([np.asarray(Su), np.asarray(Wu)[0], np.asarray(Wu)[1]]).astype(np.float32)
    Dn = np.stack([np.asarray(Sd), np.asarray(Wd)[0], np.asarray(Wd)[1]]).astype(np.float32)
    wgl = np.ascontiguousarray(
        G.reshape(3, KD, 128, MF, 128).transpose(0, 3, 2, 1, 4)
    ).reshape(3 * MF, 128, KD, 128).astype(ml_dtypes.bfloat16)
    wul = np.ascontiguousarray(
        U.reshape(3, KD, 128, MF, 128).transpose(0, 3, 2, 1, 4)
    ).reshape(3 * MF, 128, KD, 128).astype(ml_dtypes.bfloat16)
    wdl = np.ascontiguousarray(
        Dn.reshape(3, KF, 128, MD, 128).transpose(0, 3, 2, 1, 4)
    ).reshape(3 * MD, 128, KF, 128).astype(ml_dtypes.bfloat16)
    wr_h = np.ascontiguousarray(
        np.asarray(W_router, np.float32).reshape(KD, 128, 2).transpose(1, 0, 2)
    )
    rb_h = np.asarray(router_bias, np.float32).reshape(1, 2)

    in_maps = []
    for c in range(N_CORES):
        sl = slice(c * T, (c + 1) * T)
        in_maps.append({
            "xt32": np.ascontiguousarray(xt[:, sl]),
            "xtb": np.ascontiguousarray(xtb_full[:, sl]),
            "wr": wr_h,
            "rb": rb_h,
            "wgl": wgl,
            "wul": wul,
            "wdl": wdl,
        })
    return in_maps


WINDOW = 96


def pack_inputs_v2(x, W_router, router_bias, Wg, Wu, Wd, Sg, Su, Sd, T=T, D=D, F=F):
    base = pack_inputs(x, W_router, router_bias, Wg, Wu, Wd, Sg, Su, Sd, T, D, F)
    flat = np.asarray(x, np.float32).reshape(-1, D)
    flat_b = flat.astype(ml_dtypes.bfloat16)
    iota_row = np.arange(T, dtype=np.float32).reshape(1, T)
    in_maps = []
    for c, m in enumerate(base):
        m = dict(m)
        del m["xtb"]
        m["xtok"] = np.ascontiguousarray(flat_b[c * T:(c + 1) * T, :])
        m["iota"] = iota_row
        in_maps.append(m)
    return in_maps


_CACHE = {}


def _get_compiled(ver="v2"):
    key = f"nc_{ver}"
    if key not in _CACHE:
        nc = bacc.Bacc(
            "TRN2",
            target_bir_lowering=False,
            # axon clients cannot host a BassDebugger; native path can
            debug=not axon_active(),
            num_devices=N_CORES,
        )
        if ver == "v2":
            build_v2(nc, W=WINDOW)
        else:
            build(nc)
        nc.compile()
        _CACHE[key] = nc
    return _CACHE[key]


def _run_v1(np_args, x_shape, _trace=False):
    nc = _get_compiled("v1")
    in_maps = pack_inputs(*np_args)
    res = run_bass_kernel_spmd(
        nc, in_maps, core_ids=list(range(N_CORES)), trace=_trace
    )
    out_t = np.concatenate(
        [res.results[c]["yt"] for c in range(N_CORES)], axis=1
    )
    if _trace:
        _CACHE["last_result"] = res
    return np.ascontiguousarray(out_t.T).reshape(x_shape).astype(np.float32)


def kernel(x, W_router, router_bias, Wg, Wu, Wd, Sg, Su, Sd, _trace=False, **_kw):
    np_args = (x, W_router, router_bias, Wg, Wu, Wd, Sg, Su, Sd)
    x_shape = np.asarray(x).shape
    nc = _get_compiled("v2")
    in_maps = pack_inputs_v2(*np_args)
    res = run_bass_kernel_spmd(
        nc, in_maps, core_ids=list(range(N_CORES)), trace=_trace
    )
    half, w0 = T // 2, T // 2 - WINDOW // 2
    cols = []
    for c in range(N_CORES):
        c0 = int(round(float(res.results[c]["c0o"][0, 0])))
        if not (w0 <= c0 <= w0 + WINDOW):
            # expert split fell outside the static correction window
            # (~8-sigma event for these inputs): rerun with the dense kernel
            return _run_v1(np_args, x_shape, _trace)
        dest = np.rint(res.results[c]["dst"][0]).astype(np.int64)
        cols.append(res.results[c]["yt"][:, dest])  # unpermute tokens
    out_t = np.concatenate(cols, axis=1)  # [D, N]
    if _trace:
        _CACHE["last_result"] = res
    return np.ascontiguousarray(out_t.T).reshape(x_shape).astype(np.float32)
